# revision 11
# baseline (speedup 1.0000x reference)
"""Trainium2 Bass kernel for nn_CMAModel (control-fused memory attention).

Math (reference):
  q  = x @ Wq.T + ctrl @ Wc.T                  [B,T,C]
  kv = [x; fwd_mem; rev_mem]                   [B,S,C], S = T+M+R = 5440
  k  = kv @ Wk.T ; v = kv @ Wv.T
  per head h (D=128): scores = q_h k_h^T / sqrt(D), causal mask on the
  local T block only; w = softmax(scores); out_h = w_loc v_loc + gate_h *
  (w_mem v_mem); gate = sigmoid(q @ Wg.T + bg); y = concat(out_h) @ Wo.T

Sharding (8 cores, SPMD — one program, per-core behavior via input data):
  core = b*4 + g  (b = batch, g = group 0..3).  24 units of (b, head,
  T-half).  Each core runs 3 "slots": slots 0,1 = both halves of a
  "pair" head, slot 2 = one half of a "single" head (shared with the
  neighbor core).  Per batch:
    g=0: pair h0, single (h1, half A)     g=1: pair h2, single (h1, B)
    g=2: pair h3, single (h4, half A)     g=3: pair h5, single (h4, B)
  K/V are computed on-device per head-cache (cache0 = pair head,
  cache1 = single head) from the core's batch kv, column-sliced weights.

Layouts: everything feature-major ([C, tokens]) so all matmuls are
  natural (lhsT = transposed weights supplied by the host; no on-device
  transposes).  Attention uses scoresT [s, t]: softmax denominators are
  per-t sums over the s (partition) axis, computed by accumulating
  exp-tiles into a running R on DVE and one ones-vector matmul at the
  end.  Causal masking is (iota >= thr) with host-supplied per-partition
  thresholds — fully data-driven, identical control flow on all cores.

Output: per-slot out-projection partials y_p = Wo[:, h-slice].T-free
  contribution [768, 1024]; the host sums the 6 head partials per
  (batch, half) and transposes — the standard row-parallel unshard.
"""

import numpy as np

B, T, C, H, M, R = 2, 2048, 768, 6, 3072, 320
D = C // H          # 128
S = T + M + R       # 5440
P = 128
NT = (S + P - 1) // P          # 43 s-tiles (last has 64 rows)
NLOC = T // P                  # 16 local s-tiles
NCT = C // P                   # 6 feature tiles
THALF = T // 2                 # 1024
NCH = THALF // 512             # 2 chunks of 512 per half
DSCALE = float(D) ** -0.5

# per-batch slot maps: (pair_head, single_head, single_half) per group
GROUP_MAP = [(0, 1, 0), (2, 1, 1), (3, 4, 0), (5, 4, 1)]


def slot_units(g):
    hp, hs, hsh = GROUP_MAP[g]
    return [(hp, 0), (hp, 1), (hs, hsh)]


def _kchunks():
    out = []
    off = 0
    while off < S:
        w = min(512, S - off)
        out.append((off, w))
        off += w
    return out


KCH = _kchunks()               # 10x512 + 320


def build_nc(use_f32r=True, debug=False):
    import concourse.mybir as mybir
    import concourse.tile as tile
    from concourse import bacc

    f32 = mybir.dt.float32
    f32r = mybir.dt.float32r if use_f32r else f32
    AF = mybir.ActivationFunctionType
    OP = mybir.AluOpType

    mdt = f32r

    def mm(psum, lhsT, rhs, start=True, stop=True, rdt=None):
        nc.tensor.matmul(psum, lhsT, rhs, start=start, stop=stop)

    nc = bacc.Bacc("TRN2", target_bir_lowering=False, debug=False,
                   num_devices=8)

    dram = {}
    for name, shape in [
        ("kvT", [C, S]),            # batch kv, transposed
        ("xqT", [C, 3 * THALF]),    # per-slot x columns, transposed
        ("wqT", [C, 3 * P]),        # per-slot Wq head-rows, transposed
        ("wcT_s", [5, 3 * P]),      # per-slot Wc head-rows, transposed
        ("wcT", [5, C]),            # full Wc transposed
        ("wkT0", [C, P]),           # pair-head Wk rows, transposed
        ("wkT1", [C, P]),           # single-head Wk rows, transposed
        ("wvT2", [C, 2 * P]),       # [pair | single] Wv rows, transposed
        ("woT", [P, 3 * C]),        # per-slot Wo head-cols, transposed
        ("wq", [C, C]),             # Wq as-is
        ("wgT", [C, 3]),            # per-slot Wg row, transposed
        ("bg3", [1, 3]),            # per-slot gate bias
        ("ctrl5", [5, 1]),
        ("iota", [P, 512]),         # iota[i, c] = c
        ("thr", [P, 3 * NLOC * NCH]),  # causal thresholds
    ]:
        dt_ = f32r if name in ("kvT", "xqT", "wqT", "wkT0", "wkT1",
                               "wvT2", "woT") else f32
        dram[name] = nc.dram_tensor(name, shape, dt_, kind="ExternalInput")
    yp = nc.dram_tensor("yp", [3 * C, THALF], f32, kind="ExternalOutput")
    dbg = {}
    if debug:
        for name, shape in [("d_q", [P, 3 * THALF]), ("d_gate", [1, 3 * THALF]),
                            ("d_kh0", [P, 1024]), ("d_vh", [P, 512]),
                            ("d_rr", [1, 3 * THALF]),
                            ("d_att", [P, 3 * THALF])]:
            dbg[name] = nc.dram_tensor(name, shape, f32,
                                       kind="ExternalOutput")

    from contextlib import ExitStack

    with tile.TileContext(nc) as tc, ExitStack() as _ctx:
        consts = _ctx.enter_context(tc.tile_pool(name="consts", bufs=1))
        # ---- constants into SBUF ----
        wk0 = consts.tile([P, NCT, P], f32r)
        wk1 = consts.tile([P, NCT, P], f32r)
        wv2 = consts.tile([P, NCT, 2 * P], f32r)
        wqt = consts.tile([P, NCT, 3 * P], f32r)
        wgt = consts.tile([P, NCT, 3], f32)
        for ct in range(NCT):
            sl = slice(ct * P, (ct + 1) * P)
            nc.sync.dma_start(out=wk0[:, ct, :], in_=dram["wkT0"][sl, :])
            nc.sync.dma_start(out=wk1[:, ct, :], in_=dram["wkT1"][sl, :])
            nc.sync.dma_start(out=wv2[:, ct, :], in_=dram["wvT2"][sl, :])
            nc.sync.dma_start(out=wqt[:, ct, :], in_=dram["wqT"][sl, :])
            nc.sync.dma_start(out=wgt[:, ct, :], in_=dram["wgT"][sl, :])
        wot = consts.tile([P, 3 * C], f32r)
        nc.sync.dma_start(out=wot[:], in_=dram["woT"][:, :])
        wct_s = consts.tile([5, 3 * P], f32)
        nc.sync.dma_start(out=wct_s[:], in_=dram["wcT_s"][:, :])
        wct = consts.tile([5, C], f32)
        nc.sync.dma_start(out=wct[:], in_=dram["wcT"][:, :])
        bg3 = consts.tile([1, 3], f32)
        nc.sync.dma_start(out=bg3[:], in_=dram["bg3"][:, :])
        ctrl5 = consts.tile([5, 1], f32)
        nc.sync.dma_start(out=ctrl5[:], in_=dram["ctrl5"][:, :])
        iota = consts.tile([P, 512], f32)
        nc.sync.dma_start(out=iota[:], in_=dram["iota"][:, :])
        thr = consts.tile([P, 3 * NLOC * NCH], f32)
        nc.sync.dma_start(out=thr[:], in_=dram["thr"][:, :])
        ones_col = consts.tile([P, 1], f32)
        nc.vector.memset(ones_col[:], 1.0)
        ones_row = consts.tile([1, P], f32)
        nc.vector.memset(ones_row[:], 1.0)

        # ---- phase 1: tiny precomputes (plain fp32) ----
        qbs = consts.tile([P, 3], f32)      # per-slot q bias column
        qbf = consts.tile([P, NCT], f32)    # full q bias (per c-tile col)
        wfT = consts.tile([P, NCT, 3], f32r)  # fused gate weight cols
        gb3 = consts.tile([1, 3], f32)      # gate bias per slot
        with tc.tile_pool(name="p1w", bufs=1) as p1w, \
             tc.tile_pool(name="p1ps", bufs=2, space="PSUM") as p1ps:
            wqsb = p1w.tile([P, NCT, C], f32)
            for ct in range(NCT):
                nc.sync.dma_start(out=wqsb[:, ct, :],
                                  in_=dram["wq"][ct * P:(ct + 1) * P, :])
            for k in range(3):
                ps = p1ps.tile([P, 1], f32, tag="qb")
                mm(ps[:], wct_s[:, k * P:(k + 1) * P], ctrl5[:], rdt=f32)
                nc.scalar.copy(qbs[:, k:k + 1], ps[:])
            for ct in range(NCT):
                ps = p1ps.tile([P, 1], f32, tag="qb")
                mm(ps[:], wct[:, ct * P:(ct + 1) * P], ctrl5[:], rdt=f32)
                nc.scalar.copy(qbf[:, ct:ct + 1], ps[:])
            for ctp in range(NCT):
                ps = p1ps.tile([P, 3], f32, tag="wf")
                for ct in range(NCT):
                    mm(ps[:], wqsb[:, ct, ctp * P:(ctp + 1) * P],
                       wgt[:, ct, :], start=(ct == 0), stop=(ct == NCT - 1),
                       rdt=f32)
                nc.scalar.copy(wfT[:, ctp, :], ps[:])
            ps = p1ps.tile([1, 3], f32, tag="gb")
            for ct in range(NCT):
                mm(ps[:], qbf[:, ct:ct + 1], wgt[:, ct, :],
                   start=(ct == 0), stop=(ct == NCT - 1), rdt=f32)
            nc.vector.tensor_tensor(gb3[:], ps[:], bg3[:], OP.add)

        # ---- phase 2: K/V projections into SBUF caches ----
        kh0 = consts.tile([P, S], f32r)
        kh1 = consts.tile([P, S], f32r)
        vh = consts.tile([P, NT, 2 * P], f32r)
        with tc.tile_pool(name="kvp", bufs=4) as kvp, \
             tc.tile_pool(name="kvps", bufs=1, space="PSUM") as kvps:
            for sc, (off, w) in enumerate(KCH):
                pk0 = kvps.tile([P, 512], f32, tag="k0", bufs=2)
                pk1 = kvps.tile([P, 512], f32, tag="k1", bufs=2)
                subs = []
                o2 = off
                while o2 < off + w:
                    subs.append((o2 - off, min(P, off + w - o2)))
                    o2 += P
                pv = [kvps.tile([P, 2 * P], f32, tag=f"v{si}",
                                name=f"pv{si}", bufs=1)
                      for si in range(len(subs))]
                for ct in range(NCT):
                    kv_t = kvp.tile([P, 512], f32r, tag="kv")
                    nc.sync.dma_start(
                        out=kv_t[:, :w],
                        in_=dram["kvT"][ct * P:(ct + 1) * P, off:off + w])
                    mm(pk0[:, :w], wk0[:, ct, :], kv_t[:, :w],
                       start=(ct == 0), stop=(ct == NCT - 1))
                    mm(pk1[:, :w], wk1[:, ct, :], kv_t[:, :w],
                       start=(ct == 0), stop=(ct == NCT - 1))
                    for si, (so, sw) in enumerate(subs):
                        mm(pv[si][:sw, :], kv_t[:, so:so + sw],
                           wv2[:, ct, :],
                           start=(ct == 0), stop=(ct == NCT - 1))
                nc.scalar.copy(kh0[:, off:off + w], pk0[:, :w])
                nc.scalar.copy(kh1[:, off:off + w], pk1[:, :w])
                for si, (so, sw) in enumerate(subs):
                    j = (off + so) // P
                    nc.vector.tensor_copy(out=vh[:sw, j, :],
                                          in_=pv[si][:sw, :])

        # ---- phase 3: q projection + gate ----
        qsb = consts.tile([P, 3, THALF], f32r)
        gate = consts.tile([1, 3, THALF], f32)
        with tc.tile_pool(name="xqp", bufs=4) as xqp, \
             tc.tile_pool(name="qps", bufs=1, space="PSUM") as qps:
            for k in range(3):
                for ch in range(NCH):
                    pq = qps.tile([P, 512], f32, tag="q", bufs=2)
                    pg = qps.tile([1, 512], f32, tag="g", bufs=2)
                    for ct in range(NCT):
                        xq_t = xqp.tile([P, 512], f32r, tag="xq")
                        nc.sync.dma_start(
                            out=xq_t[:],
                            in_=dram["xqT"][ct * P:(ct + 1) * P,
                                            k * THALF + ch * 512:
                                            k * THALF + (ch + 1) * 512])
                        mm(pq[:], wqt[:, ct, k * P:(k + 1) * P], xq_t[:],
                           start=(ct == 0), stop=(ct == NCT - 1))
                        mm(pg[:], wfT[:, ct, k:k + 1], xq_t[:],
                           start=(ct == 0), stop=(ct == NCT - 1))
                    nc.vector.tensor_scalar_add(
                        qsb[:, k, ch * 512:(ch + 1) * 512], pq[:],
                        qbs[:, k:k + 1])
                    nc.scalar.activation(
                        gate[0:1, k, ch * 512:(ch + 1) * 512], pg[:],
                        AF.Sigmoid, bias=gb3[0:1, k:k + 1], scale=1.0)

        if debug:
            nc.sync.dma_start(out=dbg["d_q"][:, :],
                              in_=qsb[:].rearrange("p a b -> p (a b)").bitcast(f32))
            nc.sync.dma_start(out=dbg["d_gate"][0:1, :],
                              in_=gate[:].rearrange("p a b -> p (a b)"))
            nc.sync.dma_start(out=dbg["d_kh0"][:, :], in_=kh0[:, 0:1024].bitcast(f32))
            nc.sync.dma_start(out=dbg["d_vh"][:, :],
                              in_=vh[:, 0:2, :].rearrange("p a b -> p (a b)").bitcast(f32))
        # ---- phase 4: attention + output projection, per slot ----
        with tc.tile_pool(name="att", bufs=1) as att_pool, \
             tc.tile_pool(name="ep", bufs=6) as ep, \
             tc.tile_pool(name="mp", bufs=3) as mpp, \
             tc.tile_pool(name="vec", bufs=2) as vec, \
             tc.tile_pool(name="cmb", bufs=1) as cmb, \
             tc.tile_pool(name="ysb", bufs=2) as ysb, \
             tc.tile_pool(name="aps", bufs=1, space="PSUM") as aps:
            for k in range(3):
                kh = kh0 if k < 2 else kh1
                voff = 0 if k < 2 else P
                pL = [aps.tile([P, 512], f32, tag=f"l{ch}", name=f"pL{ch}")
                      for ch in range(NCH)]
                pM = [aps.tile([P, 512], f32, tag=f"m{ch}", name=f"pM{ch}")
                      for ch in range(NCH)]
                Rt = vec.tile([P, NCH, 512], f32, tag="R")
                Et = {}
                pend = []

                def emit_av(j):
                    spn = min(P, S - j * P)
                    first = j == 0 or j == NLOC
                    last = j == NLOC - 1 or j == NT - 1
                    tgt = pL if j < NLOC else pM
                    for ch in range(NCH):
                        mm(tgt[ch][:], vh[:spn, j, voff:voff + P],
                           Et.pop((j, ch))[:spn], start=first, stop=last)

                for j in range(NT):
                    spn = min(P, S - j * P)
                    for ch in range(NCH):
                        ps = aps.tile([P, 512], f32, tag="sc", bufs=2)
                        mm(ps[:spn], kh[:, j * P:j * P + spn],
                           qsb[:, k, ch * 512:(ch + 1) * 512])
                        E = ep.tile([P, 512], f32r, tag="E")
                        nc.scalar.activation(E[:spn], ps[:spn], AF.Exp,
                                             scale=DSCALE)
                        if j < NLOC:
                            col = (k * NLOC + j) * NCH + ch
                            msk = mpp.tile([P, 512], f32, tag="msk")
                            nc.vector.tensor_scalar(
                                msk[:spn], iota[:spn], thr[:spn, col:col + 1],
                                None, OP.is_ge)
                            nc.vector.tensor_tensor(E[:spn], E[:spn],
                                                    msk[:spn], OP.mult)
                        if j == 0:
                            nc.vector.tensor_copy(out=Rt[:, ch, :],
                                                  in_=E[:])
                        else:
                            nc.vector.tensor_tensor(
                                Rt[:spn, ch, :], Rt[:spn, ch, :], E[:spn],
                                OP.add)
                        Et[(j, ch)] = E
                    pend.append(j)
                    if len(pend) > 2:
                        emit_av(pend.pop(0))
                for j in pend:
                    emit_av(j)

                attb = att_pool.tile([P, NCH, 512], f32r, tag="attb")
                for ch in range(NCH):
                    pd = aps.tile([1, 512], f32, tag="sc", bufs=2)
                    mm(pd[:], ones_col[:], Rt[:, ch, :], rdt=f32)
                    rr = vec.tile([1, 512], f32, tag="rr")
                    nc.vector.reciprocal(rr[:], pd[:])
                    if debug:
                        nc.sync.dma_start(
                            out=dbg["d_rr"][0:1, k * THALF + ch * 512:
                                            k * THALF + (ch + 1) * 512],
                            in_=rr[:])
                    gr = vec.tile([1, 512], f32, tag="gr")
                    nc.vector.tensor_tensor(
                        gr[:], gate[0:1, k, ch * 512:(ch + 1) * 512], rr[:],
                        OP.mult)
                    prb = aps.tile([P, 512], f32, tag="by", bufs=2)
                    mm(prb[:], ones_row[:], rr[:], rdt=f32)
                    rb = cmb.tile([P, 512], f32, tag="rb")
                    nc.scalar.copy(rb[:], prb[:])
                    pgb = aps.tile([P, 512], f32, tag="by", bufs=2)
                    mm(pgb[:], ones_row[:], gr[:], rdt=f32)
                    gb = cmb.tile([P, 512], f32, tag="gb")
                    nc.scalar.copy(gb[:], pgb[:])
                    t1 = cmb.tile([P, 512], f32, tag="t1")
                    nc.vector.tensor_tensor(t1[:], pL[ch][:], rb[:], OP.mult)
                    t2 = cmb.tile([P, 512], f32, tag="t2")
                    nc.vector.tensor_tensor(t2[:], pM[ch][:], gb[:], OP.mult)
                    nc.vector.tensor_tensor(attb[:, ch, :], t1[:], t2[:],
                                            OP.add)
                if debug:
                    nc.sync.dma_start(
                        out=dbg["d_att"][:, k * THALF:(k + 1) * THALF],
                        in_=attb[:].rearrange("p a b -> p (a b)").bitcast(f32))
                for ot in range(NCT):
                    for ch in range(NCH):
                        py = aps.tile([P, 512], f32, tag="by", bufs=2)
                        mm(py[:], wot[:, k * C + ot * P:k * C + (ot + 1) * P],
                           attb[:, ch, :])
                        yt = ysb.tile([P, 512], f32, tag="y")
                        nc.scalar.copy(yt[:], py[:])
                        nc.sync.dma_start(
                            out=yp[k * C + ot * P:k * C + (ot + 1) * P,
                                   ch * 512:(ch + 1) * 512],
                            in_=yt[:])
    nc.compile()
    return nc


def make_in_maps(x, forward_memory, reverse_memory, ctrl, Wq, Wk, Wv, Wo,
                 Wc, Wg, bg):
    f = np.float32
    iota = np.broadcast_to(np.arange(512, dtype=f), (P, 512)).copy()
    in_maps = []
    for core in range(8):
        b, g = core // 4, core % 4
        units = slot_units(g)
        hp, hs, _ = GROUP_MAP[g]
        kv = np.concatenate(
            [x[b], forward_memory[b], reverse_memory[b]], axis=0)
        kvT = np.ascontiguousarray(kv.T, dtype=f)
        xqT = np.concatenate(
            [np.ascontiguousarray(x[b, h2 * THALF:(h2 + 1) * THALF, :].T)
             for (_, h2) in units], axis=1)
        wqT = np.concatenate(
            [np.ascontiguousarray(Wq[h * P:(h + 1) * P, :].T)
             for (h, _) in units], axis=1)
        wcT_s = np.concatenate(
            [np.ascontiguousarray(Wc[h * P:(h + 1) * P, :].T)
             for (h, _) in units], axis=1)
        wkT0 = np.ascontiguousarray(Wk[hp * P:(hp + 1) * P, :].T)
        wkT1 = np.ascontiguousarray(Wk[hs * P:(hs + 1) * P, :].T)
        wvT2 = np.concatenate(
            [np.ascontiguousarray(Wv[h * P:(h + 1) * P, :].T)
             for h in (hp, hs)], axis=1)
        woT = np.concatenate(
            [np.ascontiguousarray(Wo[:, h * P:(h + 1) * P].T)
             for (h, _) in units], axis=1)
        wgT = np.stack([Wg[h, :] for (h, _) in units], axis=1)
        bg3 = np.array([[bg[h] for (h, _) in units]], dtype=f)
        thr = np.empty((P, 3 * NLOC * NCH), dtype=f)
        i = np.arange(P, dtype=f)
        for kslot, (_, half) in enumerate(units):
            for j in range(NLOC):
                for ch in range(NCH):
                    col = (kslot * NLOC + j) * NCH + ch
                    thr[:, col] = i + 128 * j - THALF * half - 512 * ch
        in_maps.append({
            "kvT": kvT, "xqT": np.ascontiguousarray(xqT, dtype=f),
            "wqT": np.ascontiguousarray(wqT, dtype=f),
            "wcT_s": np.ascontiguousarray(wcT_s, dtype=f),
            "wcT": np.ascontiguousarray(Wc.T, dtype=f),
            "wkT0": wkT0, "wkT1": wkT1,
            "wvT2": np.ascontiguousarray(wvT2, dtype=f),
            "woT": np.ascontiguousarray(woT, dtype=f),
            "wq": np.ascontiguousarray(Wq, dtype=f),
            "wgT": np.ascontiguousarray(wgT, dtype=f),
            "bg3": bg3,
            "ctrl5": np.asarray(ctrl, dtype=f).reshape(5, 1),
            "iota": iota, "thr": thr,
        })
    return in_maps


def unshard(results):
    y = np.zeros((B, T, C), dtype=np.float32)
    for core in range(8):
        b, g = core // 4, core % 4
        ypc = results[core]["yp"]
        for kslot, (_, half) in enumerate(slot_units(g)):
            y[b, half * THALF:(half + 1) * THALF, :] += \
                ypc[kslot * C:(kslot + 1) * C, :].T
    return y


_nc_cache = {}


def _get_nc(use_f32r=True, debug=False):
    key = (use_f32r, debug)
    if key not in _nc_cache:
        _nc_cache[key] = build_nc(use_f32r, debug)
    return _nc_cache[key]


def kernel(**inputs):
    return kernel_ex(**inputs)[0]


def kernel_ex(trace=False, trace_cores=None, use_f32r=True, debug=False,
              **inputs):
    from concourse.bass_utils import run_bass_kernel_spmd

    np_inputs = {k: np.asarray(v) for k, v in inputs.items()}
    in_maps = make_in_maps(**np_inputs)
    nc = _get_nc(use_f32r, debug)
    res = run_bass_kernel_spmd(nc, in_maps, list(range(8)), trace=trace,
                               trace_cores=trace_cores)
    return unshard(res.results), res


# revision 14
# speedup vs baseline: 1.0322x; 1.0322x over previous
"""Trainium2 Bass kernel for nn_CMAModel (control-fused memory attention).

Math (reference):
  q  = x @ Wq.T + ctrl @ Wc.T                  [B,T,C]
  kv = [x; fwd_mem; rev_mem]                   [B,S,C], S = T+M+R = 5440
  k  = kv @ Wk.T ; v = kv @ Wv.T
  per head h (D=128): scores = q_h k_h^T / sqrt(D), causal mask on the
  local T block only; w = softmax(scores); out_h = w_loc v_loc + gate_h *
  (w_mem v_mem); gate = sigmoid(q @ Wg.T + bg); y = concat(out_h) @ Wo.T

Sharding (8 cores, SPMD — one program, per-core behavior via input data):
  core = b*4 + g  (b = batch, g = group 0..3).  24 units of (b, head,
  T-half).  Each core runs 3 "slots": slots 0,1 = both halves of a
  "pair" head, slot 2 = one half of a "single" head (shared with the
  neighbor core).  Per batch:
    g=0: pair h0, single (h1, half A)     g=1: pair h2, single (h1, B)
    g=2: pair h3, single (h4, half A)     g=3: pair h5, single (h4, B)
  K/V are computed on-device per head-cache (cache0 = pair head,
  cache1 = single head) from the core's batch kv, column-sliced weights.

Layouts: everything feature-major ([C, tokens]) so all matmuls are
  natural (lhsT = transposed weights supplied by the host; no on-device
  transposes).  Attention uses scoresT [s, t]: softmax denominators are
  per-t sums over the s (partition) axis, computed by accumulating
  exp-tiles into a running R on DVE and one ones-vector matmul at the
  end.  Causal masking is (iota >= thr) with host-supplied per-partition
  thresholds — fully data-driven, identical control flow on all cores.

Output: per-slot out-projection partials y_p = Wo[:, h-slice].T-free
  contribution [768, 1024]; the host sums the 6 head partials per
  (batch, half) and transposes — the standard row-parallel unshard.
"""

import numpy as np

B, T, C, H, M, R = 2, 2048, 768, 6, 3072, 320
D = C // H          # 128
S = T + M + R       # 5440
P = 128
NT = (S + P - 1) // P          # 43 s-tiles (last has 64 rows)
NLOC = T // P                  # 16 local s-tiles
NCT = C // P                   # 6 feature tiles
THALF = T // 2                 # 1024
NCH = THALF // 512             # 2 chunks of 512 per half
DSCALE = float(D) ** -0.5

# per-batch slot maps: (pair_head, single_head, single_half) per group
GROUP_MAP = [(0, 1, 0), (2, 1, 1), (3, 4, 0), (5, 4, 1)]


def slot_units(g):
    hp, hs, hsh = GROUP_MAP[g]
    return [(hp, 0), (hp, 1), (hs, hsh)]


def _kchunks():
    out = []
    off = 0
    while off < S:
        w = min(512, S - off)
        out.append((off, w))
        off += w
    return out


KCH = _kchunks()               # 10x512 + 320


def build_nc(use_f32r=True, debug=False, att_bf16=True):
    import concourse.mybir as mybir
    import concourse.tile as tile
    from concourse import bacc

    f32 = mybir.dt.float32
    f32r = mybir.dt.float32r if use_f32r else f32
    adt = mybir.dt.bfloat16 if att_bf16 else f32r
    AF = mybir.ActivationFunctionType
    OP = mybir.AluOpType

    mdt = f32r

    def mm(psum, lhsT, rhs, start=True, stop=True, rdt=None):
        nc.tensor.matmul(psum, lhsT, rhs, start=start, stop=stop)

    nc = bacc.Bacc("TRN2", target_bir_lowering=False, debug=False,
                   num_devices=8)

    dram = {}
    for name, shape in [
        ("kvT", [C, S]),            # batch kv, transposed
        ("xqT", [C, 3 * THALF]),    # per-slot x columns, transposed
        ("wqT", [C, 3 * P]),        # per-slot Wq head-rows, transposed
        ("wcT_s", [5, 3 * P]),      # per-slot Wc head-rows, transposed
        ("wcT", [5, C]),            # full Wc transposed
        ("wkT0", [C, P]),           # pair-head Wk rows, transposed
        ("wkT1", [C, P]),           # single-head Wk rows, transposed
        ("wvT2", [C, 2 * P]),       # [pair | single] Wv rows, transposed
        ("woT", [P, 3 * C]),        # per-slot Wo head-cols, transposed
        ("wq", [C, C]),             # Wq as-is
        ("wgT", [C, 3]),            # per-slot Wg row, transposed
        ("bg3", [1, 3]),            # per-slot gate bias
        ("ctrl5", [5, 1]),
        ("iota", [P, 512]),         # iota[i, c] = c
        ("thr", [P, 3 * NLOC * NCH]),  # causal thresholds
    ]:
        dt_ = f32r if name in ("kvT", "xqT", "wqT", "wkT0", "wkT1",
                               "wvT2") else f32
        if name == "woT":
            dt_ = f32 if att_bf16 else f32r
        dram[name] = nc.dram_tensor(name, shape, dt_, kind="ExternalInput")
    yp = nc.dram_tensor("yp", [3 * C, THALF], f32, kind="ExternalOutput")
    dbg = {}
    if debug:
        for name, shape in [("d_q", [P, 3 * THALF]), ("d_gate", [1, 3 * THALF]),
                            ("d_kh0", [P, 1024]), ("d_vh", [P, 512]),
                            ("d_rr", [1, 3 * THALF]),
                            ("d_att", [P, 3 * THALF])]:
            dbg[name] = nc.dram_tensor(name, shape, f32,
                                       kind="ExternalOutput")

    from contextlib import ExitStack

    with tile.TileContext(nc) as tc, ExitStack() as _ctx:
        consts = _ctx.enter_context(tc.tile_pool(name="consts", bufs=1))
        # ---- constants into SBUF ----
        wk0 = consts.tile([P, NCT, P], f32r)
        wk1 = consts.tile([P, NCT, P], f32r)
        wv2 = consts.tile([P, NCT, 2 * P], f32r)
        wqt = consts.tile([P, NCT, 3 * P], f32r)
        wgt = consts.tile([P, NCT, 3], f32)
        for ct in range(NCT):
            sl = slice(ct * P, (ct + 1) * P)
            nc.sync.dma_start(out=wk0[:, ct, :], in_=dram["wkT0"][sl, :])
            nc.sync.dma_start(out=wk1[:, ct, :], in_=dram["wkT1"][sl, :])
            nc.sync.dma_start(out=wv2[:, ct, :], in_=dram["wvT2"][sl, :])
            nc.sync.dma_start(out=wqt[:, ct, :], in_=dram["wqT"][sl, :])
            nc.sync.dma_start(out=wgt[:, ct, :], in_=dram["wgT"][sl, :])
        wot = consts.tile([P, 3 * C], adt)
        if att_bf16:
            nc.gpsimd.dma_start(out=wot[:], in_=dram["woT"][:, :])
        else:
            nc.sync.dma_start(out=wot[:], in_=dram["woT"][:, :])
        wct_s = consts.tile([5, 3 * P], f32)
        nc.sync.dma_start(out=wct_s[:], in_=dram["wcT_s"][:, :])
        wct = consts.tile([5, C], f32)
        nc.sync.dma_start(out=wct[:], in_=dram["wcT"][:, :])
        bg3 = consts.tile([1, 3], f32)
        nc.sync.dma_start(out=bg3[:], in_=dram["bg3"][:, :])
        ctrl5 = consts.tile([5, 1], f32)
        nc.sync.dma_start(out=ctrl5[:], in_=dram["ctrl5"][:, :])
        iota = consts.tile([P, 512], f32)
        nc.sync.dma_start(out=iota[:], in_=dram["iota"][:, :])
        thr = consts.tile([P, 3 * NLOC * NCH], f32)
        nc.sync.dma_start(out=thr[:], in_=dram["thr"][:, :])
        ones_col = consts.tile([P, 1], adt)
        nc.vector.memset(ones_col[:], 1.0)
        ones_row = consts.tile([1, P], f32)
        nc.vector.memset(ones_row[:], 1.0)

        # ---- phase 1: tiny precomputes (plain fp32) ----
        qbs = consts.tile([P, 3], f32)      # per-slot q bias column
        qbf = consts.tile([P, NCT], f32)    # full q bias (per c-tile col)
        wfT = consts.tile([P, NCT, 3], f32r)  # fused gate weight cols
        gb3 = consts.tile([1, 3], f32)      # gate bias per slot
        with tc.tile_pool(name="p1w", bufs=1) as p1w, \
             tc.tile_pool(name="p1ps", bufs=2, space="PSUM") as p1ps:
            wqsb = p1w.tile([P, NCT, C], f32)
            for ct in range(NCT):
                nc.sync.dma_start(out=wqsb[:, ct, :],
                                  in_=dram["wq"][ct * P:(ct + 1) * P, :])
            for k in range(3):
                ps = p1ps.tile([P, 1], f32, tag="qb")
                mm(ps[:], wct_s[:, k * P:(k + 1) * P], ctrl5[:], rdt=f32)
                nc.scalar.copy(qbs[:, k:k + 1], ps[:])
            for ct in range(NCT):
                ps = p1ps.tile([P, 1], f32, tag="qb")
                mm(ps[:], wct[:, ct * P:(ct + 1) * P], ctrl5[:], rdt=f32)
                nc.scalar.copy(qbf[:, ct:ct + 1], ps[:])
            for ctp in range(NCT):
                ps = p1ps.tile([P, 3], f32, tag="wf")
                for ct in range(NCT):
                    mm(ps[:], wqsb[:, ct, ctp * P:(ctp + 1) * P],
                       wgt[:, ct, :], start=(ct == 0), stop=(ct == NCT - 1),
                       rdt=f32)
                nc.scalar.copy(wfT[:, ctp, :], ps[:])
            ps = p1ps.tile([1, 3], f32, tag="gb")
            for ct in range(NCT):
                mm(ps[:], qbf[:, ct:ct + 1], wgt[:, ct, :],
                   start=(ct == 0), stop=(ct == NCT - 1), rdt=f32)
            nc.vector.tensor_tensor(gb3[:], ps[:], bg3[:], OP.add)

        # ---- phase 2: K/V projections into SBUF caches ----
        kh0 = consts.tile([P, S], adt)
        kh1 = consts.tile([P, S], adt)
        vh = consts.tile([P, NT, 2 * P], adt)
        with tc.tile_pool(name="kvp", bufs=4) as kvp, \
             tc.tile_pool(name="kvps", bufs=1, space="PSUM") as kvps:
            for sc, (off, w) in enumerate(KCH):
                pk0 = kvps.tile([P, 512], f32, tag="k0", bufs=2)
                pk1 = kvps.tile([P, 512], f32, tag="k1", bufs=2)
                subs = []
                o2 = off
                while o2 < off + w:
                    subs.append((o2 - off, min(P, off + w - o2)))
                    o2 += P
                pv = [kvps.tile([P, 2 * P], f32, tag=f"v{si}",
                                name=f"pv{si}", bufs=1)
                      for si in range(len(subs))]
                for ct in range(NCT):
                    kv_t = kvp.tile([P, 512], f32r, tag="kv")
                    nc.sync.dma_start(
                        out=kv_t[:, :w],
                        in_=dram["kvT"][ct * P:(ct + 1) * P, off:off + w])
                    mm(pk0[:, :w], wk0[:, ct, :], kv_t[:, :w],
                       start=(ct == 0), stop=(ct == NCT - 1))
                    mm(pk1[:, :w], wk1[:, ct, :], kv_t[:, :w],
                       start=(ct == 0), stop=(ct == NCT - 1))
                    for si, (so, sw) in enumerate(subs):
                        mm(pv[si][:sw, :], kv_t[:, so:so + sw],
                           wv2[:, ct, :],
                           start=(ct == 0), stop=(ct == NCT - 1))
                nc.vector.tensor_copy(out=kh0[:, off:off + w],
                                      in_=pk0[:, :w])
                nc.vector.tensor_copy(out=kh1[:, off:off + w],
                                      in_=pk1[:, :w])
                for si, (so, sw) in enumerate(subs):
                    j = (off + so) // P
                    nc.vector.tensor_copy(out=vh[:sw, j, :],
                                          in_=pv[si][:sw, :])

        # ---- phase 3: q projection + gate ----
        qsb = consts.tile([P, 3, THALF], adt)
        gate = consts.tile([1, 3, THALF], f32)
        with tc.tile_pool(name="xqp", bufs=4) as xqp, \
             tc.tile_pool(name="qps", bufs=1, space="PSUM") as qps:
            for k in range(3):
                for ch in range(NCH):
                    pq = qps.tile([P, 512], f32, tag="q", bufs=2)
                    pg = qps.tile([1, 512], f32, tag="g", bufs=2)
                    for ct in range(NCT):
                        xq_t = xqp.tile([P, 512], f32r, tag="xq")
                        nc.sync.dma_start(
                            out=xq_t[:],
                            in_=dram["xqT"][ct * P:(ct + 1) * P,
                                            k * THALF + ch * 512:
                                            k * THALF + (ch + 1) * 512])
                        mm(pq[:], wqt[:, ct, k * P:(k + 1) * P], xq_t[:],
                           start=(ct == 0), stop=(ct == NCT - 1))
                        mm(pg[:], wfT[:, ct, k:k + 1], xq_t[:],
                           start=(ct == 0), stop=(ct == NCT - 1))
                    nc.vector.tensor_scalar_add(
                        qsb[:, k, ch * 512:(ch + 1) * 512], pq[:],
                        qbs[:, k:k + 1])
                    nc.scalar.activation(
                        gate[0:1, k, ch * 512:(ch + 1) * 512], pg[:],
                        AF.Sigmoid, bias=gb3[0:1, k:k + 1], scale=1.0)

        if debug:
            nc.gpsimd.dma_start(out=dbg["d_q"][:, :],
                               in_=qsb[:].rearrange("p a b -> p (a b)"))
            nc.sync.dma_start(out=dbg["d_gate"][0:1, :],
                              in_=gate[:].rearrange("p a b -> p (a b)"))
            nc.gpsimd.dma_start(out=dbg["d_kh0"][:, :], in_=kh0[:, 0:1024])
            nc.gpsimd.dma_start(out=dbg["d_vh"][:, :],
                               in_=vh[:, 0:2, :].rearrange("p a b -> p (a b)"))
        # ---- phase 4: attention + output projection, per slot ----
        # Per (slot, chunk): scoresT -> exp (+ causal mask on diagonal
        # tiles) -> AV accumulation (local/memory psums) + softmax
        # denominator via an M=1 ones-matmul riding the same E tiles.
        # Slots 0/1 have compile-time-known halves, so causally-dead
        # local s-tiles are skipped outright; slot 2's half is data, so
        # it runs all local tiles with data-driven masks.
        with tc.tile_pool(name="att", bufs=1) as att_pool, \
             tc.tile_pool(name="ep", bufs=6) as ep, \
             tc.tile_pool(name="mp", bufs=3) as mpp, \
             tc.tile_pool(name="vec", bufs=2) as vec, \
             tc.tile_pool(name="cmb", bufs=1) as cmb, \
             tc.tile_pool(name="ysb", bufs=2) as ysb, \
             tc.tile_pool(name="aps", bufs=1, space="PSUM") as aps:
            for k in range(3):
                kh = kh0 if k < 2 else kh1
                voff = 0 if k < 2 else P
                attb = att_pool.tile([P, NCH, 512], adt, tag="attb")
                for ch in range(NCH):
                    if k < 2:
                        loc_end = 4 * (2 * k + ch + 1)
                        msk_lo = loc_end - 4
                    else:
                        loc_end = NLOC
                        msk_lo = 0
                    js = list(range(loc_end)) + list(range(NLOC, NT))
                    pL = aps.tile([P, 512], f32, tag="l")
                    pM = aps.tile([P, 512], f32, tag="m")
                    pden = aps.tile([1, 512], f32, tag="den")
                    qrhs = qsb[:, k, ch * 512:(ch + 1) * 512]
                    Et = {}
                    pend = []

                    def emit_av(j):
                        spn = min(P, S - j * P)
                        E = Et.pop(j)
                        tgt, first, last = (
                            (pL, j == 0, j == loc_end - 1) if j < NLOC
                            else (pM, j == NLOC, j == NT - 1))
                        mm(tgt[:], vh[:spn, j, voff:voff + P], E[:spn],
                           start=first, stop=last)
                        mm(pden[:], ones_col[:spn], E[:spn],
                           start=(j == 0), stop=(j == js[-1]))

                    for j in js:
                        spn = min(P, S - j * P)
                        ps = aps.tile([P, 512], f32, tag="sc", bufs=3)
                        mm(ps[:spn], kh[:, j * P:j * P + spn], qrhs)
                        E = ep.tile([P, 512], adt, tag="E")
                        nc.scalar.activation(E[:spn], ps[:spn], AF.Exp,
                                             scale=DSCALE)
                        if msk_lo <= j < loc_end:
                            col = (k * NLOC + j) * NCH + ch
                            msk = mpp.tile([P, 512], adt, tag="msk")
                            nc.vector.tensor_scalar(
                                msk[:spn], iota[:spn],
                                thr[:spn, col:col + 1], None, OP.is_ge)
                            nc.vector.tensor_tensor(E[:spn], E[:spn],
                                                    msk[:spn], OP.mult)
                        Et[j] = E
                        pend.append(j)
                        if len(pend) > 2:
                            emit_av(pend.pop(0))
                    for j in pend:
                        emit_av(j)

                    rr = vec.tile([1, 512], f32, tag="rr")
                    nc.vector.reciprocal(rr[:], pden[:])
                    if debug:
                        nc.sync.dma_start(
                            out=dbg["d_rr"][0:1, k * THALF + ch * 512:
                                            k * THALF + (ch + 1) * 512],
                            in_=rr[:])
                    gr = vec.tile([1, 512], f32, tag="gr")
                    nc.vector.tensor_tensor(
                        gr[:], gate[0:1, k, ch * 512:(ch + 1) * 512], rr[:],
                        OP.mult)
                    prb = aps.tile([P, 512], f32, tag="by", bufs=2)
                    mm(prb[:], ones_row[:], rr[:])
                    rb = cmb.tile([P, 512], f32, tag="rb")
                    nc.vector.tensor_copy(out=rb[:], in_=prb[:])
                    pgb = aps.tile([P, 512], f32, tag="by", bufs=2)
                    mm(pgb[:], ones_row[:], gr[:])
                    gb = cmb.tile([P, 512], f32, tag="gb")
                    nc.vector.tensor_copy(out=gb[:], in_=pgb[:])
                    t1 = cmb.tile([P, 512], f32, tag="t1")
                    nc.vector.tensor_tensor(t1[:], pL[:], rb[:], OP.mult)
                    t2 = cmb.tile([P, 512], f32, tag="t2")
                    nc.vector.tensor_tensor(t2[:], pM[:], gb[:], OP.mult)
                    nc.vector.tensor_tensor(attb[:, ch, :], t1[:], t2[:],
                                            OP.add)
                if debug:
                    nc.gpsimd.dma_start(
                        out=dbg["d_att"][:, k * THALF:(k + 1) * THALF],
                        in_=attb[:].rearrange("p a b -> p (a b)"))
                for ot in range(NCT):
                    for ch in range(NCH):
                        py = aps.tile([P, 512], f32, tag="by", bufs=2)
                        mm(py[:], wot[:, k * C + ot * P:k * C + (ot + 1) * P],
                           attb[:, ch, :])
                        yt = ysb.tile([P, 512], f32, tag="y")
                        nc.vector.tensor_copy(out=yt[:], in_=py[:])
                        nc.sync.dma_start(
                            out=yp[k * C + ot * P:k * C + (ot + 1) * P,
                                   ch * 512:(ch + 1) * 512],
                            in_=yt[:])
    nc.compile()
    return nc


def make_in_maps(x, forward_memory, reverse_memory, ctrl, Wq, Wk, Wv, Wo,
                 Wc, Wg, bg):
    f = np.float32
    iota = np.broadcast_to(np.arange(512, dtype=f), (P, 512)).copy()
    in_maps = []
    for core in range(8):
        b, g = core // 4, core % 4
        units = slot_units(g)
        hp, hs, _ = GROUP_MAP[g]
        kv = np.concatenate(
            [x[b], forward_memory[b], reverse_memory[b]], axis=0)
        kvT = np.ascontiguousarray(kv.T, dtype=f)
        xqT = np.concatenate(
            [np.ascontiguousarray(x[b, h2 * THALF:(h2 + 1) * THALF, :].T)
             for (_, h2) in units], axis=1)
        wqT = np.concatenate(
            [np.ascontiguousarray(Wq[h * P:(h + 1) * P, :].T)
             for (h, _) in units], axis=1)
        wcT_s = np.concatenate(
            [np.ascontiguousarray(Wc[h * P:(h + 1) * P, :].T)
             for (h, _) in units], axis=1)
        wkT0 = np.ascontiguousarray(Wk[hp * P:(hp + 1) * P, :].T)
        wkT1 = np.ascontiguousarray(Wk[hs * P:(hs + 1) * P, :].T)
        wvT2 = np.concatenate(
            [np.ascontiguousarray(Wv[h * P:(h + 1) * P, :].T)
             for h in (hp, hs)], axis=1)
        woT = np.concatenate(
            [np.ascontiguousarray(Wo[:, h * P:(h + 1) * P].T)
             for (h, _) in units], axis=1)
        wgT = np.stack([Wg[h, :] for (h, _) in units], axis=1)
        bg3 = np.array([[bg[h] for (h, _) in units]], dtype=f)
        thr = np.empty((P, 3 * NLOC * NCH), dtype=f)
        i = np.arange(P, dtype=f)
        for kslot, (_, half) in enumerate(units):
            for j in range(NLOC):
                for ch in range(NCH):
                    col = (kslot * NLOC + j) * NCH + ch
                    thr[:, col] = i + 128 * j - THALF * half - 512 * ch
        in_maps.append({
            "kvT": kvT, "xqT": np.ascontiguousarray(xqT, dtype=f),
            "wqT": np.ascontiguousarray(wqT, dtype=f),
            "wcT_s": np.ascontiguousarray(wcT_s, dtype=f),
            "wcT": np.ascontiguousarray(Wc.T, dtype=f),
            "wkT0": wkT0, "wkT1": wkT1,
            "wvT2": np.ascontiguousarray(wvT2, dtype=f),
            "woT": np.ascontiguousarray(woT, dtype=f),
            "wq": np.ascontiguousarray(Wq, dtype=f),
            "wgT": np.ascontiguousarray(wgT, dtype=f),
            "bg3": bg3,
            "ctrl5": np.asarray(ctrl, dtype=f).reshape(5, 1),
            "iota": iota, "thr": thr,
        })
    return in_maps


def unshard(results):
    y = np.zeros((B, T, C), dtype=np.float32)
    for core in range(8):
        b, g = core // 4, core % 4
        ypc = results[core]["yp"]
        for kslot, (_, half) in enumerate(slot_units(g)):
            y[b, half * THALF:(half + 1) * THALF, :] += \
                ypc[kslot * C:(kslot + 1) * C, :].T
    return y


_nc_cache = {}


def _get_nc(use_f32r=True, debug=False, att_bf16=True):
    key = (use_f32r, debug, att_bf16)
    if key not in _nc_cache:
        _nc_cache[key] = build_nc(use_f32r, debug, att_bf16)
    return _nc_cache[key]


def kernel(**inputs):
    return kernel_ex(**inputs)[0]


def kernel_ex(trace=False, trace_cores=None, use_f32r=True, debug=False,
              att_bf16=True, **inputs):
    from concourse.bass_utils import run_bass_kernel_spmd

    np_inputs = {k: np.asarray(v) for k, v in inputs.items()}
    in_maps = make_in_maps(**np_inputs)
    nc = _get_nc(use_f32r, debug, att_bf16)
    res = run_bass_kernel_spmd(nc, in_maps, list(range(8)), trace=trace,
                               trace_cores=trace_cores)
    return unshard(res.results), res
